# revision 33
# baseline (speedup 1.0000x reference)
"""DeepseekV2 MLA attention fusion on 8 Trainium2 NeuronCores.

Collective-free strategy (collectives in this environment cost ~50ms each
through the emulated runtime -- 290ms of the 316ms baseline):
  - Every core receives the FULL hidden_states (transposed, bf16) and
    redundantly computes the rank-space a-projections + rmsnorms + k_pe
    rope for all T (17.7 GMAC, ~0.45ms on the PE -- cheap enough to
    replicate 8x rather than pay for one AllGather).
  - Each core then owns 4 of the 32 heads: q/kv up-projections, rope on
    q_pe, causal attention, and a PARTIAL output projection: its heads'
    512 rows of w_o against ALL 4096 output columns.
  - The 8 partial [T, HID] f32 outputs are summed on the host (the
    all-reduce after o_proj moves off-device).

Layout: everything on-device is "features-on-partitions, T-on-free"
(transposed) so no on-device transposes are needed. Attention:
scores^T[k,q] on the PE (causal blocks only), exp on ScalarE with the
softmax scale folded in (no max subtraction -- scores are O(10) here so
exp is safe in fp32), lower-triangle mask on diagonal blocks, P@V plus
an all-ones matmul for the row sums accumulated in PSUM.
"""

import numpy as np
import ml_dtypes

import concourse.bass as bass
import concourse.mybir as mybir
import concourse.tile as tile
from concourse import bacc
from concourse.masks import make_identity, make_upper_triangular

T = 2048
HID = 4096
NH = 32
DN = 128
DR = 64
DV = 128
QLR = 1536
KVLR = 512
EPS = 1e-6
THETA = 10000.0
SCALE = float((DN + DR) ** -0.5)

NCORES = 8
HL = NH // NCORES          # 4 heads per core
FQ = QLR // 128            # 12 qlr chunks
FKV = KVLR // 128          # 4 kvlr chunks
KH = HID // 128            # 32 hid chunks
MA = 17                    # a-proj M tiles (2176 = 17*128, zero padded)
NT = T // 128              # 16 T tiles
NCH = T // 512             # 4 column chunks of 512
CA = 512                   # phase-A T-chunk width

BF = mybir.dt.bfloat16
F32 = mybir.dt.float32
NPBF = ml_dtypes.bfloat16


def build_module(n_rep: int = 1, upto: str = "D"):
    """Build the Bass module (same program for every core)."""
    nc = bacc.Bacc("TRN2", target_bir_lowering=False, debug=False,
                   num_devices=NCORES)

    # all host-side layouts are partition-major so every DMA moves long
    # contiguous per-partition runs (no on-the-fly gather/rearrange)
    hsT = nc.dram_tensor("hsT", [NCH, 128, KH, CA], BF, kind="ExternalInput")
    wa = nc.dram_tensor("wa", [MA, 128, KH, 128], BF, kind="ExternalInput")
    wqb = nc.dram_tensor("wqb", [6, 128, FQ, 128], BF, kind="ExternalInput")
    wkn = nc.dram_tensor("wkn", [HL, 128, FKV, 128], BF, kind="ExternalInput")
    wv = nc.dram_tensor("wv", [128, FKV, HL * DV], BF, kind="ExternalInput")
    wo = nc.dram_tensor("wo", [128, HL, HID], BF, kind="ExternalInput")
    cosq = nc.dram_tensor("cosq", [128, T], BF, kind="ExternalInput")
    sgnsinq = nc.dram_tensor("sgnsinq", [128, T], BF, kind="ExternalInput")
    out_o = nc.dram_tensor("out_o", [T, HID], BF, kind="ExternalOutput")

    with tile.TileContext(nc) as tc:
        with tc.tile_pool(name="const", bufs=1) as const_pool:
            ones_bf = const_pool.tile([128, 128], BF)
            nc.vector.memset(ones_bf, 1.0)
            # mask-as-matmul: sc += negtri.T @ id128 adds -1e9 above the
            # causal diagonal, so exp() output is already masked and the
            # DVE mask-mul drops out of the PE->Act->DVE->PE chain
            negtri = const_pool.tile([128, 128], BF)
            make_upper_triangular(nc, negtri[:], val=-1e9, diag=False)
            id128 = const_pool.tile([128, 128], BF)
            make_identity(nc, id128[:])
            # rope swap as a permutation matmul: swp row i = e_{sigma(i)}
            swp = const_pool.tile([128, 128], BF)
            for half in range(4):
                a, b = 32 * half, 32 * (half + 1)
                s0 = b if half % 2 == 0 else a - 32
                nc.sync.dma_start(out=swp[a:b, :], in_=id128[s0:s0 + 32, :])
            eps_sb = const_pool.tile([128, 1], F32)
            nc.vector.memset(eps_sb, EPS)
            cosq_sb = const_pool.tile([128, T], BF)
            nc.sync.dma_start(out=cosq_sb, in_=cosq.ap())
            sgnsinq_sb = const_pool.tile([128, T], BF)
            nc.sync.dma_start(out=sgnsinq_sb, in_=sgnsinq.ap())

            for _rep in range(n_rep):
                _body(nc, tc, hsT, wa, wqb, wkn, wv, wo, out_o,
                      ones_bf, negtri, id128, swp, eps_sb, cosq_sb,
                      sgnsinq_sb, upto)

    nc.compile()
    return nc


def _body(nc, tc, hsT, wa, wqb, wkn, wv, wo, out_o,
          ones_bf, negtri, id128, swp, eps_sb, cosq_sb, sgnsinq_sb,
          upto="D"):
    from contextlib import ExitStack

    def dbg_drain(pool, src_ap, n):
        dbg = pool.tile([128, n], BF, tag="dbg", name="dbg")
        nc.vector.tensor_copy(out=dbg[:], in_=src_ap)
        nc.sync.dma_start(out=out_o.ap()[0:128, 0:n], in_=dbg[:])

    with ExitStack() as phases:
        persist = phases.enter_context(tc.tile_pool(name="persist", bufs=1))
        # k_pe rope, duplicated into both 64-row halves so each head of a
        # rope-pair tile can read a partition-aligned copy in phase C
        kpe_sb = persist.tile([128, T], BF, tag="kpe", name="kpe")

        # qcT/kvcT live phase A -> end of phase B, then manually freed so
        # the attention/o-proj phases get their SBUF back (stack allocator).
        actx = phases.enter_context(ExitStack())
        acts = actx.enter_context(tc.tile_pool(name="acts", bufs=1))
        qcT_sb = acts.tile([128, FQ, T], BF, tag="qcT", name="qcT")
        kvcT_sb = acts.tile([128, FKV, T], BF, tag="kvcT", name="kvcT")

        # ---------------- Phase A: a-projections + rmsnorm + k_pe rope ----
        # Full T on every core, processed as 2 pairs of 512-col chunks.
        # Each wa m-tile is loaded once per PAIR (halves weight traffic) and
        # its stationary serves two back-to-back matmuls per k (the real HW
        # hides weight loads only when the stationary is reused).
        with ExitStack() as pa:
            hs_pool = pa.enter_context(tc.tile_pool(name="hsA", bufs=2))
            wa_pool = pa.enter_context(tc.tile_pool(name="waA", bufs=2))
            psA = pa.enter_context(
                tc.tile_pool(name="psA", bufs=2, space="PSUM"))
            psR = pa.enter_context(
                tc.tile_pool(name="psR", bufs=4, space="PSUM"))
            rawA = pa.enter_context(tc.tile_pool(name="rawA", bufs=17))
            sqA = pa.enter_context(tc.tile_pool(name="sqA", bufs=2))
            ropeA = pa.enter_context(tc.tile_pool(name="ropeA", bufs=1))

            for cp in range(NCH // 2):
                c0, c1 = 2 * cp, 2 * cp + 1
                hs0 = hs_pool.tile([128, KH, CA], BF, tag="hs",
                                   name=f"hs{c0}")
                nc.sync.dma_start(out=hs0, in_=hsT.ap()[c0])
                hs1 = hs_pool.tile([128, KH, CA], BF, tag="hs",
                                   name=f"hs{c1}")
                nc.sync.dma_start(out=hs1, in_=hsT.ap()[c1])

                rs_q = [psR.tile([128, CA], F32, tag="rs", name=f"rsq{c}")
                        for c in (c0, c1)]
                rs_kv = [psR.tile([128, CA], F32, tag="rs", name=f"rskv{c}")
                         for c in (c0, c1)]
                raws = []
                for m in range(MA):
                    wa_sb = wa_pool.tile([128, KH, 128], BF, tag="wa")
                    nc.sync.dma_start(out=wa_sb, in_=wa.ap()[m])
                    ps = psA.tile([128, 2, CA], F32)   # 2 PSUM banks
                    for k in range(KH):
                        nc.tensor.matmul(ps[:, 0, :], wa_sb[:, k, :],
                                         hs0[:, k, :],
                                         start=(k == 0), stop=(k == KH - 1))
                        nc.tensor.matmul(ps[:, 1, :], wa_sb[:, k, :],
                                         hs1[:, k, :],
                                         start=(k == 0), stop=(k == KH - 1))
                    raw = rawA.tile([128, 2, CA], BF, tag="raw",
                                    name=f"raw{m}")
                    # drain on ScalarE: keeps PSUM turnover off the DVE,
                    # whose pair-end rmsnorm burst otherwise stalls the PE
                    nc.scalar.activation(raw[:], ps[:],
                                         mybir.ActivationFunctionType.Copy)
                    raws.append(raw)
                    if m < FQ + FKV:
                        sq = sqA.tile([128, 2, CA], BF, tag="sq")
                        nc.scalar.activation(
                            sq[:], ps[:],
                            mybir.ActivationFunctionType.Square)
                        for i in range(2):
                            dst = rs_q[i] if m < FQ else rs_kv[i]
                            nc.tensor.matmul(
                                dst[:], ones_bf[:], sq[:, i, :],
                                start=(m in (0, FQ)),
                                stop=(m in (FQ - 1, FQ + FKV - 1)))

                for i, c in enumerate((c0, c1)):
                    cs = slice(CA * c, CA * (c + 1))
                    # rsqrt(mean + eps), broadcast across partitions already
                    rq = sqA.tile([128, CA], F32, tag="rq", bufs=1)
                    nc.scalar.activation(rq[:], rs_q[i][:],
                                         mybir.ActivationFunctionType.Sqrt,
                                         bias=eps_sb[:], scale=1.0 / QLR)
                    nc.vector.reciprocal(rq[:], rq[:])
                    rkv = sqA.tile([128, CA], F32, tag="rkv", bufs=1)
                    nc.scalar.activation(rkv[:], rs_kv[i][:],
                                         mybir.ActivationFunctionType.Sqrt,
                                         bias=eps_sb[:], scale=1.0 / KVLR)
                    nc.vector.reciprocal(rkv[:], rkv[:])

                    for m in range(FQ):
                        nc.vector.tensor_mul(qcT_sb[:, m, cs],
                                             raws[m][:, i, :], rq[:])
                    for m in range(FKV):
                        nc.vector.tensor_mul(kvcT_sb[:, m, cs],
                                             raws[FQ + m][:, i, :], rkv[:])

                    # k_pe rope. raws[16] rows 0:64 = [x1;x2]; rows 64:128
                    # = [x2;x1] (host packed swapped weight columns), so one
                    # 64-row partition-move DMA aligns the swap.
                    kpe_raw = raws[16]
                    kswap = ropeA.tile([64, CA], BF, tag="kswap", bufs=2)
                    nc.sync.dma_start(out=kswap[0:64, :],
                                      in_=kpe_raw[64:128, i, :])
                    ku = ropeA.tile([64, CA], BF, tag="ku", bufs=2)
                    kw = ropeA.tile([64, CA], BF, tag="kw", bufs=2)
                    nc.vector.tensor_mul(ku[:], kpe_raw[0:64, i, :],
                                         cosq_sb[0:64, cs])
                    nc.vector.tensor_mul(kw[:], kswap[:],
                                         sgnsinq_sb[0:64, cs])
                    nc.vector.tensor_add(kpe_sb[0:64, cs], ku[:], kw[:])
                    # duplicate into rows 64:128 (partition move -> DMA)
                    nc.sync.dma_start(out=kpe_sb[64:128, cs],
                                      in_=kpe_sb[0:64, cs])

            if upto == "A":
                dbg_drain(ropeA, qcT_sb[:, 0, 0:CA], CA)
                return

        # ---------------- Phase B: up-projections + q rope ----------------
        # attention-phase operands (stay alive through phase C)
        bout = phases.enter_context(
            tc.tile_pool(name="bout", bufs=1, side="right"))
        qn_sb = [bout.tile([128, T], BF, tag=f"qn{h}", name=f"qn{h}")
                 for h in range(HL)]
        # roped q_pe kept as pair tiles: head 2p in rows 0:64, 2p+1 in 64:128
        rp_sb = [bout.tile([128, T], BF, tag=f"rp{i}", name=f"rp{i}")
                 for i in range(HL // 2)]
        kn_sb = [bout.tile([128, T], BF, tag=f"kn{h}", name=f"kn{h}")
                 for h in range(HL)]
        v_sb = [bout.tile([128, HL * DV], BF, tag=f"v{j}", name=f"v{j}")
                for j in range(NT)]

        with ExitStack() as pb:
            wB_pool = pb.enter_context(tc.tile_pool(name="wB", bufs=1))
            psB = pb.enter_context(
                tc.tile_pool(name="psB", bufs=4, space="PSUM"))
            ropeB = pb.enter_context(tc.tile_pool(name="ropeB", bufs=2))

            wqb_sb = wB_pool.tile([128, 6, FQ, 128], BF)
            nc.sync.dma_start(out=wqb_sb,
                              in_=wqb.ap().rearrange("m p k q -> p m k q"))
            wkn_sb = wB_pool.tile([128, HL, FKV, 128], BF, tag="wkn")
            nc.sync.dma_start(out=wkn_sb,
                              in_=wkn.ap().rearrange("m p k q -> p m k q"))
            wv_sb = wB_pool.tile([128, FKV, HL * DV], BF, tag="wv")
            nc.sync.dma_start(out=wv_sb, in_=wv.ap())

            # q/k up-projections over T column-chunk PAIRS: each stationary
            # weight block serves two back-to-back matmuls (HW weight-load
            # reuse), accumulating into a 2-bank PSUM tile
            for cp in range(NCH // 2):
                css = [slice(512 * c, 512 * (c + 1))
                       for c in (2 * cp, 2 * cp + 1)]
                for m in range(6):
                    ps2 = psB.tile([128, 2, 512], F32, tag="ps2",
                                   bufs=2)
                    for kc in range(FQ):
                        for i in range(2):
                            nc.tensor.matmul(ps2[:, i, :],
                                             wqb_sb[:, m, kc, :],
                                             qcT_sb[:, kc, css[i]],
                                             start=(kc == 0),
                                             stop=(kc == FQ - 1))
                    for i in range(2):
                        cs = css[i]
                        if m < HL:
                            nc.vector.tensor_copy(out=qn_sb[m][:, cs],
                                                  in_=ps2[:, i, :])
                        else:
                            # rope pair tile (two heads of 64 rows each);
                            # 32-row swap via permutation matmul
                            pair = m - HL
                            qraw = ropeB.tile([128, 512], BF, tag="qraw")
                            nc.vector.tensor_copy(out=qraw[:],
                                                  in_=ps2[:, i, :])
                            qsw_ps = psB.tile([128, 512], F32, tag="qswp",
                                              bufs=2)
                            nc.tensor.matmul(qsw_ps[:], swp[:], qraw[:],
                                             start=True, stop=True)
                            qu = ropeB.tile([128, 512], F32, tag="qu")
                            qw = ropeB.tile([128, 512], F32, tag="qw")
                            nc.vector.tensor_mul(qu[:], qraw[:],
                                                 cosq_sb[:, cs])
                            nc.vector.tensor_mul(qw[:], qsw_ps[:],
                                                 sgnsinq_sb[:, cs])
                            nc.vector.tensor_add(rp_sb[pair][:, cs],
                                                 qu[:], qw[:])

                # k_nope for this column-chunk pair
                for m in range(HL):
                    ps2 = psB.tile([128, 2, 512], F32, tag="ps2",
                                   bufs=2)
                    for kc in range(FKV):
                        for i in range(2):
                            nc.tensor.matmul(ps2[:, i, :],
                                             wkn_sb[:, m, kc, :],
                                             kvcT_sb[:, kc, css[i]],
                                             start=(kc == 0),
                                             stop=(kc == FKV - 1))
                    for i in range(2):
                        nc.vector.tensor_copy(out=kn_sb[m][:, css[i]],
                                              in_=ps2[:, i, :])

            # v (natural layout): one T-tile at a time
            for j in range(NT):
                ps = psB.tile([128, 512], F32, tag="ps", bufs=2)
                for kc in range(FKV):
                    nc.tensor.matmul(ps[:],
                                     kvcT_sb[:, kc, 128 * j:128 * (j + 1)],
                                     wv_sb[:, kc, :],
                                     start=(kc == 0), stop=(kc == FKV - 1))
                nc.vector.tensor_copy(out=v_sb[j][:], in_=ps[:])

            if upto == "B":
                dbg_drain(ropeB, v_sb[0][:], 512)
                return

        actx.close()  # free qcT/kvcT

        # prefetch the o-proj weights during phase C so phase D never waits
        woP = phases.enter_context(tc.tile_pool(name="woP", bufs=1))
        wo_sb = woP.tile([128, HL, HID], BF)
        nc.sync.dma_start(out=wo_sb, in_=wo.ap())

        # ---------------- Phase C: attention ------------------------------
        atP = phases.enter_context(
            tc.tile_pool(name="atP", bufs=1, side="right"))
        attn_sb = [atP.tile([128, T], BF, tag=f"at{h}", name=f"at{h}")
                   for h in range(HL)]

        with ExitStack() as pc:
            psSC = pc.enter_context(
                tc.tile_pool(name="psSC", bufs=4, space="PSUM"))
            psAT = pc.enter_context(
                tc.tile_pool(name="psAT", bufs=1, space="PSUM"))
            psSM = pc.enter_context(
                tc.tile_pool(name="psSM", bufs=1, space="PSUM"))
            pP = pc.enter_context(tc.tile_pool(name="pP", bufs=6))
            recP = pc.enter_context(tc.tile_pool(name="recP", bufs=2))

            # column-chunk PAIRS per head: each kn/kpe/v stationary block
            # serves both chunks back-to-back (HW weight-load reuse)
            for h in range(HL):
                rb = 64 * (h % 2)   # row base of this head in the pair tiles
                qpe = rp_sb[h // 2]
                for cp in range(NCH // 2):
                    cc = (2 * cp, 2 * cp + 1)
                    attn_ps2 = psAT.tile([128, 2, 512], F32)
                    sums_ps2 = psSM.tile([128, 2, 512], F32)
                    for j in range(4 * cc[1] + 4):
                        # active chunks of this pair, with per-chunk geometry
                        act = []
                        for i in range(2):
                            c = cc[i]
                            if j > 4 * c + 3:
                                continue
                            off = max(0, 128 * j - 512 * c)
                            act.append((i, c, off, j >= 4 * c, 4 * c + 3))
                        js = slice(128 * j, 128 * (j + 1))
                        scs = {}
                        for i, c, off, diag, jmax in act:
                            sc = psSC.tile([128, 512], F32)
                            scs[i] = sc
                            nc.tensor.matmul(
                                sc[:, off:], kn_sb[h][:, js],
                                qn_sb[h][:, 512 * c + off:512 * (c + 1)],
                                start=True, stop=False)
                        for i, c, off, diag, jmax in act:
                            if diag:
                                # -1e9 above the diagonal, via the PE
                                nc.tensor.matmul(
                                    scs[i][:, off:off + 128], negtri[:],
                                    id128[:], start=False, stop=False)
                        for i, c, off, diag, jmax in act:
                            nc.tensor.matmul(
                                scs[i][:, off:],
                                kpe_sb[rb:rb + 64, js],
                                qpe[rb:rb + 64, 512 * c + off:512 * (c + 1)],
                                start=False, stop=True)
                        ps_ = {}
                        for i, c, off, diag, jmax in act:
                            p_sb = pP.tile([128, 512], BF)
                            ps_[i] = p_sb
                            nc.scalar.activation(
                                p_sb[:, off:], scs[i][:, off:],
                                mybir.ActivationFunctionType.Exp,
                                scale=SCALE)
                        for i, c, off, diag, jmax in act:
                            nc.tensor.matmul(
                                attn_ps2[:, i, off:],
                                v_sb[j][:, DV * h:DV * (h + 1)],
                                ps_[i][:, off:],
                                start=(j == 0), stop=(j == jmax))
                        for i, c, off, diag, jmax in act:
                            nc.tensor.matmul(
                                sums_ps2[:, i, off:], ones_bf[:],
                                ps_[i][:, off:],
                                start=(j == 0), stop=(j == jmax))
                    for i in range(2):
                        rec = recP.tile([128, 512], F32)
                        nc.vector.reciprocal(rec[:], sums_ps2[:, i, :])
                        nc.vector.tensor_mul(
                            attn_sb[h][:, 512 * cc[i]:512 * (cc[i] + 1)],
                            attn_ps2[:, i, :], rec[:])

            if upto == "C":
                dbg_drain(recP, attn_sb[0][:, 0:512], 512)
                return

        # ---------------- Phase D: partial output projection --------------
        # out[T, HID] = sum_h attn[h]^T @ w_o[head rows, :]; accumulate the
        # HL local heads in PSUM, 8 banks = one full 4096-wide T-tile row.
        with ExitStack() as pd:
            psO = pd.enter_context(
                tc.tile_pool(name="psO", bufs=8, space="PSUM"))
            oP = pd.enter_context(tc.tile_pool(name="oP", bufs=4))

            for t in range(NT):
                pss = [psO.tile([128, 512], F32, tag="pso",
                                name=f"pso{t}_{cc}") for cc in range(8)]
                for h in range(HL):
                    for cc in range(8):
                        nc.tensor.matmul(
                            pss[cc][:], attn_sb[h][:, 128 * t:128 * (t + 1)],
                            wo_sb[:, h, 512 * cc:512 * (cc + 1)],
                            start=(h == 0), stop=(h == HL - 1))
                o_sb = oP.tile([128, HID], BF, tag="osb", name="osb")
                for cc in range(8):
                    if cc % 2 == 0:
                        nc.vector.tensor_copy(
                            out=o_sb[:, 512 * cc:512 * (cc + 1)],
                            in_=pss[cc][:])
                    else:
                        nc.scalar.activation(
                            o_sb[:, 512 * cc:512 * (cc + 1)], pss[cc][:],
                            mybir.ActivationFunctionType.Copy)
                nc.sync.dma_start(
                    out=out_o.ap()[128 * t:128 * (t + 1), :], in_=o_sb[:])


# ---------------------------------------------------------------------------
# Host side
# ---------------------------------------------------------------------------

_ROPE_PERM = np.concatenate([np.arange(0, DR, 2), np.arange(1, DR, 2)])


def _prepare_inputs(positions, hidden_states, w_qa, w_kva, g_qa, w_qb,
                    g_kva, w_kvb, w_o):
    """Build the 8 per-core input dicts (numpy, host-side layout prep)."""
    positions = np.asarray(positions)
    hs = np.asarray(hidden_states, dtype=np.float32)
    w_qa = np.asarray(w_qa, np.float32)
    w_kva = np.asarray(w_kva, np.float32)
    # rmsnorm(y, g) @ W == rmsnorm_nogain(y) @ (g[:, None] * W)
    w_qb = np.asarray(w_qb, np.float32) * np.asarray(
        g_qa, np.float32)[:, None]
    w_kvb = np.asarray(w_kvb, np.float32) * np.asarray(
        g_kva, np.float32)[:, None]
    w_o = np.asarray(w_o, np.float32)

    # full hidden_states, transposed, chunk-major: [NCH, 128, KH, CA]
    hsT_full = np.ascontiguousarray(
        hs.T.reshape(KH, 128, NCH, CA).transpose(2, 1, 0, 3)).astype(NPBF)

    # a-projection weights: [w_qa | w_kva_c | w_kva_pe(perm) | pe swapped]
    # the swapped copy fills the otherwise-wasted rows 64:128 of the m=16
    # output tile, so the rope swap needs only one partition-move DMA
    wa_full = np.zeros((HID, MA * 128), np.float32)
    wa_full[:, :QLR] = w_qa
    wa_full[:, QLR:QLR + KVLR] = w_kva[:, :KVLR]
    pe_cols = w_kva[:, KVLR:][:, _ROPE_PERM]
    wa_full[:, QLR + KVLR:QLR + KVLR + DR] = pe_cols
    wa_full[:, QLR + KVLR + DR:QLR + KVLR + 2 * DR] = np.concatenate(
        [pe_cols[:, 32:], pe_cols[:, :32]], axis=1)
    wa_t = np.ascontiguousarray(
        wa_full.reshape(KH, 128, MA, 128).transpose(2, 1, 0, 3)
    ).astype(NPBF)  # [MA, 128, KH, 128]

    # rope tables
    inv_freq = (1.0 / (THETA ** (np.arange(0, DR, 2, dtype=np.float32) / DR))
                ).astype(np.float32)
    f = positions.astype(np.float32)[:, None] * inv_freq[None, :]  # [T, 32]
    cos = np.cos(f).astype(np.float32).T  # [32, T]
    sin = np.sin(f).astype(np.float32).T
    cosq128 = np.tile(cos, (4, 1)).astype(NPBF)
    sgnsinq128 = np.concatenate([-sin, sin, -sin, sin],
                                axis=0).astype(NPBF)

    w_qb3 = w_qb.reshape(QLR, NH, DN + DR)
    w_kvb3 = w_kvb.reshape(KVLR, NH, DN + DV)

    in_maps = []
    for d in range(NCORES):
        heads = range(HL * d, HL * (d + 1))

        # q b-proj columns: 4 nope blocks then 2 rope pair blocks
        cols = [w_qb3[:, h, :DN] for h in heads]
        for pair in range(2):
            h0 = HL * d + 2 * pair
            cols.append(w_qb3[:, h0, DN:][:, _ROPE_PERM])
            cols.append(w_qb3[:, h0 + 1, DN:][:, _ROPE_PERM])
        wqb_local = np.concatenate(cols, axis=1)  # [1536, 768]
        wqb_t = np.ascontiguousarray(
            wqb_local.reshape(FQ, 128, 6, 128).transpose(2, 1, 0, 3)
        ).astype(NPBF)

        wkn_local = np.concatenate(
            [w_kvb3[:, h, :DN] for h in heads], axis=1)  # [512, 512]
        wkn_t = np.ascontiguousarray(
            wkn_local.reshape(FKV, 128, HL, 128).transpose(2, 1, 0, 3)
        ).astype(NPBF)

        wv_local = np.concatenate(
            [w_kvb3[:, h, DN:] for h in heads], axis=1)  # [512, 512]
        wv_t = np.ascontiguousarray(
            wv_local.reshape(FKV, 128, HL * DV).transpose(1, 0, 2)
        ).astype(NPBF)

        # o-proj rows for this core's heads, ALL output columns
        wo_local = np.ascontiguousarray(
            w_o[512 * d:512 * (d + 1), :].reshape(HL, 128, HID)
            .transpose(1, 0, 2)).astype(NPBF)

        in_maps.append({
            "hsT": hsT_full,
            "wa": wa_t,
            "wqb": wqb_t,
            "wkn": wkn_t,
            "wv": wv_t,
            "wo": wo_local,
            "cosq": cosq128,
            "sgnsinq": sgnsinq128,
        })
    return in_maps


_CACHED_NC = {}


def _get_module(n_rep=1, upto="D"):
    key = (n_rep, upto)
    if key not in _CACHED_NC:
        _CACHED_NC[key] = build_module(n_rep, upto)
    return _CACHED_NC[key]


def run(in_maps, n_rep=1, upto="D", **kwargs):
    from concourse.bass_utils import run_bass_kernel_spmd
    nc = _get_module(n_rep, upto)
    return run_bass_kernel_spmd(nc, in_maps, core_ids=list(range(NCORES)),
                                **kwargs)


_CACHED_RUNNER = {}


def device_runner(in_maps, n_rep=1, upto="D", nc=None, cache_key=None):
    """Zero-transfer executor for timing: jit built once, inputs resident
    on device, each call executes the NEFF on all 8 cores and blocks.

    run_bass_kernel_spmd (the axon path) rebuilds jax.jit(shard_map(...))
    and re-transfers ~300MB of inputs EVERY call, so wall-differencing it
    measures mostly host/tunnel overhead that scales with NEFF size. This
    runner removes all per-call host work except dispatch.
    """
    import jax
    from jax.sharding import Mesh, NamedSharding, PartitionSpec
    from jax.experimental.shard_map import shard_map
    from concourse import bass2jax

    key = cache_key if cache_key is not None else (n_rep, upto)
    if key in _CACHED_RUNNER:
        return _CACHED_RUNNER[key]

    if nc is None:
        nc = _get_module(n_rep, upto)
    bass2jax.install_neuronx_cc_hook()

    partition_name = (nc.partition_id_tensor.name
                      if nc.partition_id_tensor else None)
    in_names, out_names, out_avals, zero_outs = [], [], [], []
    for alloc in nc.m.functions[0].allocations:
        if not isinstance(alloc, mybir.MemoryLocationSet):
            continue
        name = alloc.memorylocations[0].name
        if alloc.kind == "ExternalInput":
            if name != partition_name:
                in_names.append(name)
        elif alloc.kind == "ExternalOutput":
            shape = tuple(alloc.tensor_shape)
            dtype = mybir.dt.np(alloc.dtype)
            out_names.append(name)
            out_avals.append(jax.core.ShapedArray(shape, dtype))
            zero_outs.append(np.zeros(shape, dtype))
    n_params = len(in_names)
    bind_names = list(in_names) + list(out_names)
    if partition_name is not None:
        bind_names.append(partition_name)

    def _body(*args):
        operands = list(args)
        if partition_name is not None:
            operands.append(bass2jax.partition_id_tensor())
        outs = bass2jax._bass_exec_p.bind(
            *operands,
            out_avals=tuple(out_avals),
            in_names=tuple(bind_names),
            out_names=tuple(out_names),
            lowering_input_output_aliases=(),
            sim_require_finite=True,
            sim_require_nnan=True,
            nc=nc,
        )
        return tuple(outs)

    devices = jax.devices()[:NCORES]
    mesh = Mesh(np.asarray(devices), ("core",))
    in_specs = (PartitionSpec("core"),) * (n_params + len(out_names))
    out_specs = (PartitionSpec("core"),) * len(out_names)
    fn = jax.jit(shard_map(_body, mesh=mesh, in_specs=in_specs,
                           out_specs=out_specs, check_rep=False),
                 keep_unused=True)  # no donation: buffers reused across calls

    sh = NamedSharding(mesh, PartitionSpec("core"))
    per_core = [[np.asarray(m[name]) for name in in_names] for m in in_maps]
    dev_in = [jax.device_put(
        np.concatenate([per_core[c][i] for c in range(NCORES)], axis=0), sh)
        for i in range(n_params)]
    dev_zero = [jax.device_put(
        np.zeros((NCORES * z.shape[0], *z.shape[1:]), z.dtype), sh)
        for z in zero_outs]

    def call():
        out = fn(*dev_in, *dev_zero)
        jax.block_until_ready(out)
        return out

    call()  # warm: trace + compile + first exec
    _CACHED_RUNNER[key] = call
    return call


def kernel(**inputs):
    in_maps = _prepare_inputs(**inputs)
    res = run(in_maps)
    out = res.results[0]["out_o"].astype(np.float32)
    for d in range(1, NCORES):
        out += res.results[d]["out_o"]
    return out



# revision 39
# speedup vs baseline: 1.0608x; 1.0608x over previous
"""DeepseekV2 MLA attention fusion on 8 Trainium2 NeuronCores.

Collective-free strategy (collectives in this environment cost ~50ms each
through the emulated runtime):
  - Every core receives the FULL hidden_states (transposed, bf16) and
    redundantly computes the rank-space a-projections + rmsnorms + k_pe
    rope for all T (17.7 GMAC on the PE -- cheap enough to replicate 8x
    rather than pay for one AllGather).
  - Each core then owns 4 of the 32 heads: q/kv up-projections, rope on
    q_pe, causal attention, and a PARTIAL output projection: its heads'
    512 rows of w_o against ALL 4096 output columns.
  - The 8 partial [T, HID] bf16 outputs are summed on the host (the
    all-reduce after o_proj moves off-device).

Layout: everything on-device is "features-on-partitions, T-on-free"
(transposed) so no on-device transposes are needed; every DRAM layout is
partition-major so all DMAs move long contiguous per-partition runs.
Attention: scores^T[k,q] on the PE (causal blocks only), exp on ScalarE
with the softmax scale folded in (no max subtraction -- scores are O(10)
here so exp is safe in fp32), lower-triangle mask on diagonal blocks,
P@V plus an all-ones matmul for the row sums accumulated in PSUM.

Timing learnings (HW tracks TimelineSim * ~1.19 very consistently):
  - PSUM drains on ScalarE, rmsnorm scaling on DVE: keeps PSUM turnover
    off the DVE whose chunk-end bursts otherwise stall the PE.
  - Loop restructures for stationary weight reuse do NOT pay on HW
    (tried: paired column chunks everywhere; HW mirrored the sim's
    serialization cost with zero reuse benefit).
  - Mask-as-matmul / rope-swap-as-matmul: no HW benefit over the DVE
    mask mul and 4 small partition-move DMAs; costs its extra PE time.
  - bf16 rope tables + partial outputs are well within the 2e-2 budget
    (total rel err ~7.2e-3, bf16-dominated).
"""

import numpy as np
import ml_dtypes

import concourse.bass as bass
import concourse.mybir as mybir
import concourse.tile as tile
from concourse import bacc
from concourse.masks import make_upper_triangular

T = 2048
HID = 4096
NH = 32
DN = 128
DR = 64
DV = 128
QLR = 1536
KVLR = 512
EPS = 1e-6
THETA = 10000.0
SCALE = float((DN + DR) ** -0.5)

NCORES = 8
HL = NH // NCORES          # 4 heads per core
FQ = QLR // 128            # 12 qlr chunks
FKV = KVLR // 128          # 4 kvlr chunks
KH = HID // 128            # 32 hid chunks
MA = 17                    # a-proj M tiles (2176 = 17*128, zero padded)
NT = T // 128              # 16 T tiles
NCH = T // 512             # 4 column chunks of 512
CA = 512                   # phase-A T-chunk width

BF = mybir.dt.bfloat16
F32 = mybir.dt.float32
NPBF = ml_dtypes.bfloat16


def build_module(n_rep: int = 1, upto: str = "D"):
    """Build the Bass module (same program for every core)."""
    nc = bacc.Bacc("TRN2", target_bir_lowering=False, debug=False,
                   num_devices=NCORES)

    # all host-side layouts are partition-major so every DMA moves long
    # contiguous per-partition runs (no on-the-fly gather/rearrange)
    hsT = nc.dram_tensor("hsT", [NCH, 128, KH, CA], BF, kind="ExternalInput")
    wa = nc.dram_tensor("wa", [MA, 128, KH, 128], BF, kind="ExternalInput")
    wqb = nc.dram_tensor("wqb", [6, 128, FQ, 128], BF, kind="ExternalInput")
    wkn = nc.dram_tensor("wkn", [HL, 128, FKV, 128], BF, kind="ExternalInput")
    wv = nc.dram_tensor("wv", [128, FKV, HL * DV], BF, kind="ExternalInput")
    wo = nc.dram_tensor("wo", [128, HL, HID], BF, kind="ExternalInput")
    cosq = nc.dram_tensor("cosq", [128, T], BF, kind="ExternalInput")
    sgnsinq = nc.dram_tensor("sgnsinq", [128, T], BF, kind="ExternalInput")
    out_o = nc.dram_tensor("out_o", [T, HID], BF, kind="ExternalOutput")

    with tile.TileContext(nc) as tc:
        with tc.tile_pool(name="const", bufs=1) as const_pool:
            ones_bf = const_pool.tile([128, 128], BF)
            nc.vector.memset(ones_bf, 1.0)
            trimask = const_pool.tile([128, 128], BF)
            make_upper_triangular(nc, trimask[:], val=1.0, diag=True)
            eps_sb = const_pool.tile([128, 1], F32)
            nc.vector.memset(eps_sb, EPS)
            cosq_sb = const_pool.tile([128, T], BF)
            nc.sync.dma_start(out=cosq_sb, in_=cosq.ap())
            sgnsinq_sb = const_pool.tile([128, T], BF)
            nc.sync.dma_start(out=sgnsinq_sb, in_=sgnsinq.ap())

            for _rep in range(n_rep):
                _body(nc, tc, hsT, wa, wqb, wkn, wv, wo, out_o,
                      ones_bf, trimask, eps_sb, cosq_sb, sgnsinq_sb, upto)

    nc.compile()
    return nc


def _body(nc, tc, hsT, wa, wqb, wkn, wv, wo, out_o,
          ones_bf, trimask, eps_sb, cosq_sb, sgnsinq_sb, upto="D"):
    from contextlib import ExitStack

    def dbg_drain(pool, src_ap, n):
        dbg = pool.tile([128, n], BF, tag="dbg", name="dbg")
        nc.vector.tensor_copy(out=dbg[:], in_=src_ap)
        nc.sync.dma_start(out=out_o.ap()[0:128, 0:n], in_=dbg[:])

    with ExitStack() as phases:
        persist = phases.enter_context(tc.tile_pool(name="persist", bufs=1))
        # k_pe rope, duplicated into both 64-row halves so each head of a
        # rope-pair tile can read a partition-aligned copy in phase C
        kpe_sb = persist.tile([128, T], BF, tag="kpe", name="kpe")

        # qcT/kvcT live phase A -> end of phase B, then manually freed so
        # the attention/o-proj phases get their SBUF back (stack allocator).
        actx = phases.enter_context(ExitStack())
        acts = actx.enter_context(tc.tile_pool(name="acts", bufs=1))
        qcT_sb = acts.tile([128, FQ, T], BF, tag="qcT", name="qcT")
        kvcT_sb = acts.tile([128, FKV, T], BF, tag="kvcT", name="kvcT")

        # ---------------- Phase A: a-projections + rmsnorm + k_pe rope ----
        # Full T on every core, chunked into NCH passes of CA columns.
        with ExitStack() as pa:
            hs_pool = pa.enter_context(tc.tile_pool(name="hsA", bufs=2))
            wa_pool = pa.enter_context(tc.tile_pool(name="waA", bufs=4))
            psA = pa.enter_context(
                tc.tile_pool(name="psA", bufs=2, space="PSUM"))
            psR = pa.enter_context(
                tc.tile_pool(name="psR", bufs=1, space="PSUM"))
            rawA = pa.enter_context(tc.tile_pool(name="rawA", bufs=17))
            sqA = pa.enter_context(tc.tile_pool(name="sqA", bufs=3))
            ropeA = pa.enter_context(tc.tile_pool(name="ropeA", bufs=1))

            for c in range(NCH):
                cs = slice(CA * c, CA * (c + 1))
                hs_sb = hs_pool.tile([128, KH, CA], BF, tag="hs")
                nc.sync.dma_start(out=hs_sb, in_=hsT.ap()[c])

                rs_q = psR.tile([128, CA], F32, tag="rsq")
                rs_kv = psR.tile([128, CA], F32, tag="rskv")
                raws = []
                for m in range(MA):
                    wa_sb = wa_pool.tile([128, KH, 128], BF, tag="wa")
                    nc.sync.dma_start(out=wa_sb, in_=wa.ap()[m])
                    ps = psA.tile([128, CA], F32)
                    for k in range(KH):
                        nc.tensor.matmul(ps[:], wa_sb[:, k, :],
                                         hs_sb[:, k, :],
                                         start=(k == 0), stop=(k == KH - 1))
                    raw = rawA.tile([128, CA], BF, tag="raw",
                                    name=f"raw{m}")
                    # drain on ScalarE: keeps PSUM turnover off the DVE,
                    # whose chunk-end rmsnorm burst otherwise stalls the PE
                    nc.scalar.activation(raw[:], ps[:],
                                         mybir.ActivationFunctionType.Copy)
                    raws.append(raw)
                    if m < FQ + FKV:
                        sq = sqA.tile([128, CA], BF, tag="sq")
                        nc.scalar.activation(
                            sq[:], ps[:],
                            mybir.ActivationFunctionType.Square)
                        if m < FQ:
                            nc.tensor.matmul(rs_q[:], ones_bf[:], sq[:],
                                             start=(m == 0),
                                             stop=(m == FQ - 1))
                        else:
                            nc.tensor.matmul(rs_kv[:], ones_bf[:], sq[:],
                                             start=(m == FQ),
                                             stop=(m == FQ + FKV - 1))

                # rsqrt(mean + eps), broadcast across partitions already
                rq = sqA.tile([128, CA], F32, tag="rq", bufs=1)
                nc.scalar.activation(rq[:], rs_q[:],
                                     mybir.ActivationFunctionType.Sqrt,
                                     bias=eps_sb[:], scale=1.0 / QLR)
                nc.vector.reciprocal(rq[:], rq[:])
                rkv = sqA.tile([128, CA], F32, tag="rkv", bufs=1)
                nc.scalar.activation(rkv[:], rs_kv[:],
                                     mybir.ActivationFunctionType.Sqrt,
                                     bias=eps_sb[:], scale=1.0 / KVLR)
                nc.vector.reciprocal(rkv[:], rkv[:])

                for m in range(FQ):
                    nc.vector.tensor_mul(qcT_sb[:, m, cs], raws[m][:], rq[:])
                for m in range(FKV):
                    nc.vector.tensor_mul(kvcT_sb[:, m, cs],
                                         raws[FQ + m][:], rkv[:])

                # k_pe rope. raws[16] rows 0:64 = [x1;x2]; rows 64:128 =
                # [x2;x1] (host packed swapped weight columns there), so one
                # 64-row partition-move DMA aligns the swap.
                kpe_raw = raws[16]
                kswap = ropeA.tile([64, CA], BF, tag="kswap")
                nc.sync.dma_start(out=kswap[0:64, :], in_=kpe_raw[64:128, :])
                ku = ropeA.tile([64, CA], BF, tag="ku")
                nc.vector.tensor_mul(ku[:], kpe_raw[0:64, :],
                                     cosq_sb[0:64, cs])
                nc.vector.tensor_mul(kswap[:], kswap[:],
                                     sgnsinq_sb[0:64, cs])
                nc.vector.tensor_add(kpe_sb[0:64, cs], ku[:],
                                     kswap[:])
                # duplicate into rows 64:128 (partition move -> DMA)
                nc.sync.dma_start(out=kpe_sb[64:128, cs],
                                  in_=kpe_sb[0:64, cs])

            if upto == "A":
                dbg_drain(ropeA, qcT_sb[:, 0, 0:CA], CA)
                return

        # ---------------- Phase B: up-projections + q rope ----------------
        # attention-phase operands (stay alive through phase C)
        bout = phases.enter_context(
            tc.tile_pool(name="bout", bufs=1, side="right"))
        qn_sb = [bout.tile([128, T], BF, tag=f"qn{h}", name=f"qn{h}")
                 for h in range(HL)]
        # roped q_pe kept as pair tiles: head 2p in rows 0:64, 2p+1 in 64:128
        rp_sb = [bout.tile([128, T], BF, tag=f"rp{i}", name=f"rp{i}")
                 for i in range(HL // 2)]
        kn_sb = [bout.tile([128, T], BF, tag=f"kn{h}", name=f"kn{h}")
                 for h in range(HL)]
        v_sb = [bout.tile([128, HL * DV], BF, tag=f"v{j}", name=f"v{j}")
                for j in range(NT)]

        with ExitStack() as pb:
            wB_pool = pb.enter_context(tc.tile_pool(name="wB", bufs=1))
            psB = pb.enter_context(
                tc.tile_pool(name="psB", bufs=4, space="PSUM"))
            ropeB = pb.enter_context(tc.tile_pool(name="ropeB", bufs=2))

            wqb_sb = wB_pool.tile([128, 6, FQ, 128], BF)
            nc.sync.dma_start(out=wqb_sb,
                              in_=wqb.ap().rearrange("m p k q -> p m k q"))
            wkn_sb = wB_pool.tile([128, HL, FKV, 128], BF, tag="wkn")
            nc.sync.dma_start(out=wkn_sb,
                              in_=wkn.ap().rearrange("m p k q -> p m k q"))
            wv_sb = wB_pool.tile([128, FKV, HL * DV], BF, tag="wv")
            nc.sync.dma_start(out=wv_sb, in_=wv.ap())

            # q up-projection, chunk by chunk over T columns
            for c in range(NCH):
                cs = slice(512 * c, 512 * (c + 1))
                for m in range(6):
                    ps = psB.tile([128, 512], F32, tag="ps")
                    for kc in range(FQ):
                        nc.tensor.matmul(ps[:], wqb_sb[:, m, kc, :],
                                         qcT_sb[:, kc, cs],
                                         start=(kc == 0), stop=(kc == FQ - 1))
                    if m < HL:
                        nc.vector.tensor_copy(out=qn_sb[m][:, cs], in_=ps[:])
                    else:
                        # rope pair tile (two heads of 64 rows each).
                        pair = m - HL
                        qraw = ropeB.tile([128, 512], F32, tag="qraw")
                        nc.vector.tensor_copy(out=qraw[:], in_=ps[:])
                        qsw = ropeB.tile([128, 512], F32, tag="qsw")
                        for half in range(4):
                            a, b = 32 * half, 32 * (half + 1)
                            s0 = b if half % 2 == 0 else a - 32
                            nc.sync.dma_start(out=qsw[a:b, :],
                                              in_=qraw[s0:s0 + 32, :])
                        qu = ropeB.tile([128, 512], F32, tag="qu")
                        qw = ropeB.tile([128, 512], F32, tag="qw")
                        nc.vector.tensor_mul(qu[:], qraw[:], cosq_sb[:, cs])
                        nc.vector.tensor_mul(qw[:], qsw[:], sgnsinq_sb[:, cs])
                        nc.vector.tensor_add(rp_sb[pair][:, cs],
                                             qu[:], qw[:])

                # k_nope for this column chunk
                for m in range(HL):
                    ps = psB.tile([128, 512], F32, tag="ps")
                    for kc in range(FKV):
                        nc.tensor.matmul(ps[:], wkn_sb[:, m, kc, :],
                                         kvcT_sb[:, kc, cs],
                                         start=(kc == 0),
                                         stop=(kc == FKV - 1))
                    nc.vector.tensor_copy(out=kn_sb[m][:, cs], in_=ps[:])

            # v (natural layout): one T-tile at a time
            for j in range(NT):
                ps = psB.tile([128, 512], F32, tag="ps")
                for kc in range(FKV):
                    nc.tensor.matmul(ps[:],
                                     kvcT_sb[:, kc, 128 * j:128 * (j + 1)],
                                     wv_sb[:, kc, :],
                                     start=(kc == 0), stop=(kc == FKV - 1))
                nc.vector.tensor_copy(out=v_sb[j][:], in_=ps[:])

            if upto == "B":
                dbg_drain(ropeB, v_sb[0][:], 512)
                return

        actx.close()  # free qcT/kvcT

        # prefetch the o-proj weights during phase C so phase D never waits
        woP = phases.enter_context(tc.tile_pool(name="woP", bufs=1))
        wo_sb = woP.tile([128, HL, HID], BF)
        nc.sync.dma_start(out=wo_sb, in_=wo.ap())

        # ---------------- Phase C: attention ------------------------------
        atP = phases.enter_context(
            tc.tile_pool(name="atP", bufs=1, side="right"))
        attn_sb = [atP.tile([128, T], BF, tag=f"at{h}", name=f"at{h}")
                   for h in range(HL)]

        with ExitStack() as pc:
            psSC = pc.enter_context(
                tc.tile_pool(name="psSC", bufs=3, space="PSUM"))
            psAT = pc.enter_context(
                tc.tile_pool(name="psAT", bufs=2, space="PSUM"))
            psSM = pc.enter_context(
                tc.tile_pool(name="psSM", bufs=2, space="PSUM"))
            pP = pc.enter_context(tc.tile_pool(name="pP", bufs=6))
            recP = pc.enter_context(tc.tile_pool(name="recP", bufs=2))

            for h in range(HL):
                rb = 64 * (h % 2)   # row base of this head in the pair tiles
                qpe = rp_sb[h // 2]
                for c in range(NCH):
                    attn_ps = psAT.tile([128, 512], F32)
                    sums_ps = psSM.tile([128, 512], F32)
                    jmax = 4 * c + 3
                    for j in range(jmax + 1):
                        off = max(0, 128 * j - 512 * c)
                        sc = psSC.tile([128, 512], F32)
                        nc.tensor.matmul(
                            sc[:, off:], kn_sb[h][:, 128 * j:128 * (j + 1)],
                            qn_sb[h][:, 512 * c + off:512 * (c + 1)],
                            start=True, stop=False)
                        nc.tensor.matmul(
                            sc[:, off:],
                            kpe_sb[rb:rb + 64, 128 * j:128 * (j + 1)],
                            qpe[rb:rb + 64, 512 * c + off:512 * (c + 1)],
                            start=False, stop=True)
                        p_sb = pP.tile([128, 512], BF)
                        nc.scalar.activation(p_sb[:, off:], sc[:, off:],
                                             mybir.ActivationFunctionType.Exp,
                                             scale=SCALE)
                        if j >= 4 * c:
                            nc.vector.tensor_mul(p_sb[:, off:off + 128],
                                                 p_sb[:, off:off + 128],
                                                 trimask[:])
                        nc.tensor.matmul(attn_ps[:, off:],
                                         v_sb[j][:, DV * h:DV * (h + 1)],
                                         p_sb[:, off:],
                                         start=(j == 0), stop=(j == jmax))
                        nc.tensor.matmul(sums_ps[:, off:], ones_bf[:],
                                         p_sb[:, off:],
                                         start=(j == 0), stop=(j == jmax))
                    rec = recP.tile([128, 512], F32)
                    nc.vector.reciprocal(rec[:], sums_ps[:])
                    nc.vector.tensor_mul(
                        attn_sb[h][:, 512 * c:512 * (c + 1)],
                        attn_ps[:], rec[:])

            if upto == "C":
                dbg_drain(recP, attn_sb[0][:, 0:512], 512)
                return

        # ---------------- Phase D: partial output projection --------------
        # out[T, HID] = sum_h attn[h]^T @ w_o[head rows, :]; accumulate the
        # HL local heads in PSUM, 8 banks = one full 4096-wide T-tile row.
        with ExitStack() as pd:
            psO = pd.enter_context(
                tc.tile_pool(name="psO", bufs=8, space="PSUM"))
            oP = pd.enter_context(tc.tile_pool(name="oP", bufs=4))

            for t in range(NT):
                pss = [psO.tile([128, 512], F32, tag="pso",
                                name=f"pso{t}_{cc}") for cc in range(8)]
                for h in range(HL):
                    for cc in range(8):
                        nc.tensor.matmul(
                            pss[cc][:], attn_sb[h][:, 128 * t:128 * (t + 1)],
                            wo_sb[:, h, 512 * cc:512 * (cc + 1)],
                            start=(h == 0), stop=(h == HL - 1))
                o_sb = oP.tile([128, HID], BF, tag="osb", name="osb")
                for cc in range(8):
                    if cc % 2 == 0:
                        nc.vector.tensor_copy(
                            out=o_sb[:, 512 * cc:512 * (cc + 1)],
                            in_=pss[cc][:])
                    else:
                        nc.scalar.activation(
                            o_sb[:, 512 * cc:512 * (cc + 1)], pss[cc][:],
                            mybir.ActivationFunctionType.Copy)
                nc.sync.dma_start(
                    out=out_o.ap()[128 * t:128 * (t + 1), :], in_=o_sb[:])


# ---------------------------------------------------------------------------
# Host side
# ---------------------------------------------------------------------------

_ROPE_PERM = np.concatenate([np.arange(0, DR, 2), np.arange(1, DR, 2)])


def _prepare_inputs(positions, hidden_states, w_qa, w_kva, g_qa, w_qb,
                    g_kva, w_kvb, w_o):
    """Build the 8 per-core input dicts (numpy, host-side layout prep)."""
    positions = np.asarray(positions)
    hs = np.asarray(hidden_states, dtype=np.float32)
    w_qa = np.asarray(w_qa, np.float32)
    w_kva = np.asarray(w_kva, np.float32)
    # rmsnorm(y, g) @ W == rmsnorm_nogain(y) @ (g[:, None] * W)
    w_qb = np.asarray(w_qb, np.float32) * np.asarray(
        g_qa, np.float32)[:, None]
    w_kvb = np.asarray(w_kvb, np.float32) * np.asarray(
        g_kva, np.float32)[:, None]
    w_o = np.asarray(w_o, np.float32)

    # full hidden_states, transposed, chunk-major: [NCH, 128, KH, CA]
    hsT_full = np.ascontiguousarray(
        hs.T.reshape(KH, 128, NCH, CA).transpose(2, 1, 0, 3)).astype(NPBF)

    # a-projection weights: [w_qa | w_kva_c | w_kva_pe(perm) | pe swapped]
    # the swapped copy fills the otherwise-wasted rows 64:128 of the m=16
    # output tile, so the rope swap needs only one partition-move DMA
    wa_full = np.zeros((HID, MA * 128), np.float32)
    wa_full[:, :QLR] = w_qa
    wa_full[:, QLR:QLR + KVLR] = w_kva[:, :KVLR]
    pe_cols = w_kva[:, KVLR:][:, _ROPE_PERM]
    wa_full[:, QLR + KVLR:QLR + KVLR + DR] = pe_cols
    wa_full[:, QLR + KVLR + DR:QLR + KVLR + 2 * DR] = np.concatenate(
        [pe_cols[:, 32:], pe_cols[:, :32]], axis=1)
    wa_t = np.ascontiguousarray(
        wa_full.reshape(KH, 128, MA, 128).transpose(2, 1, 0, 3)
    ).astype(NPBF)  # [MA, 128, KH, 128]

    # rope tables
    inv_freq = (1.0 / (THETA ** (np.arange(0, DR, 2, dtype=np.float32) / DR))
                ).astype(np.float32)
    f = positions.astype(np.float32)[:, None] * inv_freq[None, :]  # [T, 32]
    cos = np.cos(f).astype(np.float32).T  # [32, T]
    sin = np.sin(f).astype(np.float32).T
    cosq128 = np.tile(cos, (4, 1)).astype(NPBF)
    sgnsinq128 = np.concatenate([-sin, sin, -sin, sin],
                                axis=0).astype(NPBF)

    w_qb3 = w_qb.reshape(QLR, NH, DN + DR)
    w_kvb3 = w_kvb.reshape(KVLR, NH, DN + DV)

    in_maps = []
    for d in range(NCORES):
        heads = range(HL * d, HL * (d + 1))

        # q b-proj columns: 4 nope blocks then 2 rope pair blocks
        cols = [w_qb3[:, h, :DN] for h in heads]
        for pair in range(2):
            h0 = HL * d + 2 * pair
            cols.append(w_qb3[:, h0, DN:][:, _ROPE_PERM])
            cols.append(w_qb3[:, h0 + 1, DN:][:, _ROPE_PERM])
        wqb_local = np.concatenate(cols, axis=1)  # [1536, 768]
        wqb_t = np.ascontiguousarray(
            wqb_local.reshape(FQ, 128, 6, 128).transpose(2, 1, 0, 3)
        ).astype(NPBF)

        wkn_local = np.concatenate(
            [w_kvb3[:, h, :DN] for h in heads], axis=1)  # [512, 512]
        wkn_t = np.ascontiguousarray(
            wkn_local.reshape(FKV, 128, HL, 128).transpose(2, 1, 0, 3)
        ).astype(NPBF)

        wv_local = np.concatenate(
            [w_kvb3[:, h, DN:] for h in heads], axis=1)  # [512, 512]
        wv_t = np.ascontiguousarray(
            wv_local.reshape(FKV, 128, HL * DV).transpose(1, 0, 2)
        ).astype(NPBF)

        # o-proj rows for this core's heads, ALL output columns
        wo_local = np.ascontiguousarray(
            w_o[512 * d:512 * (d + 1), :].reshape(HL, 128, HID)
            .transpose(1, 0, 2)).astype(NPBF)

        in_maps.append({
            "hsT": hsT_full,
            "wa": wa_t,
            "wqb": wqb_t,
            "wkn": wkn_t,
            "wv": wv_t,
            "wo": wo_local,
            "cosq": cosq128,
            "sgnsinq": sgnsinq128,
        })
    return in_maps


_CACHED_NC = {}


def _get_module(n_rep=1, upto="D"):
    key = (n_rep, upto)
    if key not in _CACHED_NC:
        _CACHED_NC[key] = build_module(n_rep, upto)
    return _CACHED_NC[key]


def run(in_maps, n_rep=1, upto="D", **kwargs):
    from concourse.bass_utils import run_bass_kernel_spmd
    nc = _get_module(n_rep, upto)
    return run_bass_kernel_spmd(nc, in_maps, core_ids=list(range(NCORES)),
                                **kwargs)


_CACHED_RUNNER = {}


def device_runner(in_maps, n_rep=1, upto="D", nc=None, cache_key=None):
    """Zero-transfer executor for timing: jit built once, inputs resident
    on device, each call executes the NEFF on all 8 cores and blocks.

    run_bass_kernel_spmd (the axon path) rebuilds jax.jit(shard_map(...))
    and re-transfers ~300MB of inputs EVERY call, so wall-differencing it
    measures mostly host/tunnel overhead that scales with NEFF size. This
    runner removes all per-call host work except dispatch.
    """
    import jax
    from jax.sharding import Mesh, NamedSharding, PartitionSpec
    from jax.experimental.shard_map import shard_map
    from concourse import bass2jax

    key = cache_key if cache_key is not None else (n_rep, upto)
    if key in _CACHED_RUNNER:
        return _CACHED_RUNNER[key]

    if nc is None:
        nc = _get_module(n_rep, upto)
    bass2jax.install_neuronx_cc_hook()

    partition_name = (nc.partition_id_tensor.name
                      if nc.partition_id_tensor else None)
    in_names, out_names, out_avals, zero_outs = [], [], [], []
    for alloc in nc.m.functions[0].allocations:
        if not isinstance(alloc, mybir.MemoryLocationSet):
            continue
        name = alloc.memorylocations[0].name
        if alloc.kind == "ExternalInput":
            if name != partition_name:
                in_names.append(name)
        elif alloc.kind == "ExternalOutput":
            shape = tuple(alloc.tensor_shape)
            dtype = mybir.dt.np(alloc.dtype)
            out_names.append(name)
            out_avals.append(jax.core.ShapedArray(shape, dtype))
            zero_outs.append(np.zeros(shape, dtype))
    n_params = len(in_names)
    bind_names = list(in_names) + list(out_names)
    if partition_name is not None:
        bind_names.append(partition_name)

    def _body(*args):
        operands = list(args)
        if partition_name is not None:
            operands.append(bass2jax.partition_id_tensor())
        outs = bass2jax._bass_exec_p.bind(
            *operands,
            out_avals=tuple(out_avals),
            in_names=tuple(bind_names),
            out_names=tuple(out_names),
            lowering_input_output_aliases=(),
            sim_require_finite=True,
            sim_require_nnan=True,
            nc=nc,
        )
        return tuple(outs)

    devices = jax.devices()[:NCORES]
    mesh = Mesh(np.asarray(devices), ("core",))
    in_specs = (PartitionSpec("core"),) * (n_params + len(out_names))
    out_specs = (PartitionSpec("core"),) * len(out_names)
    fn = jax.jit(shard_map(_body, mesh=mesh, in_specs=in_specs,
                           out_specs=out_specs, check_rep=False),
                 keep_unused=True)  # no donation: buffers reused across calls

    sh = NamedSharding(mesh, PartitionSpec("core"))
    per_core = [[np.asarray(m[name]) for name in in_names] for m in in_maps]
    dev_in = [jax.device_put(
        np.concatenate([per_core[c][i] for c in range(NCORES)], axis=0), sh)
        for i in range(n_params)]
    dev_zero = [jax.device_put(
        np.zeros((NCORES * z.shape[0], *z.shape[1:]), z.dtype), sh)
        for z in zero_outs]

    def call():
        out = fn(*dev_in, *dev_zero)
        jax.block_until_ready(out)
        return out

    call()  # warm: trace + compile + first exec
    _CACHED_RUNNER[key] = call
    return call


def kernel(**inputs):
    in_maps = _prepare_inputs(**inputs)
    res = run(in_maps)
    out = res.results[0]["out_o"].astype(np.float32)
    for d in range(1, NCORES):
        out += res.results[d]["out_o"].astype(np.float32)
    return out
